# revision 11
# baseline (speedup 1.0000x reference)
"""Trainium2 Bass kernel for the EnergyConstrainedPredictiveCodingModel.

Pure data parallel: batch (8192) sharded across 8 NeuronCores (1024 rows
each); weights replicated. Everything on-device is kept in transposed layout
[feature, batch] so the PE array contracts along the partition dim:

    Y.T[N, B] = W @ X.T  ==  matmul(out, lhsT=W.T[K, N], rhs=X.T[K, B])

Matmuls run in float32r (full-rate fp32 PE path). Reference scale factors
(0.1 i2t, -1/1.2 vip, 0.02 t2z, 0.5 theta, eps_zhat/1.2) are folded into
host-side preprocessing. softplus = Ln(Exp(x)+1) on ACT; the kernel is
structured so ACT needs exactly two table sets (natural_log_exp, sigmoid).
Losses are reduced on-chip into per-partition partials (acc) and finished on
the host. Small weights are packed into one DRAM tensor; xh/xz/xez/xeh are
packed into one; z_hat/h into one output.
"""

import os

import numpy as np

import concourse.bacc as bacc
import concourse.bass as bass
import concourse.tile as tile
from concourse import mybir
from concourse.bass_utils import run_bass_kernel_spmd

f32 = mybir.dt.float32
f32r = mybir.dt.float32r
Alu = mybir.AluOpType
Act = mybir.ActivationFunctionType

N_CORES = 8
B = 8192
BC = B // N_CORES  # 1024 batch rows per core
D_IN, D_Z, D_H, D_T, D_REC = 1024, 256, 256, 16, 256
P = 128
KI = D_IN // P   # 8 k-tiles for D_IN
KZ = D_Z // P    # 2 k-tiles for D_Z / D_H / D_REC
HF = 512         # matmul moving-dim half (fp32 max 512)
NH = BC // HF    # 2 halves

# packed-weight layout: name -> (K, N); stored as [128, K//128 * N] per weight
WSPEC = [
    ("pmu", D_H, D_Z), ("plv", D_H, D_Z), ("i2t", D_IN, D_T),
    ("vip", D_Z, D_T), ("qmu", D_IN, D_Z), ("qlv", D_IN, D_Z),
    ("r1", D_Z, D_REC), ("r2", D_REC, D_IN), ("z2h", D_Z, D_H),
    ("h2h", D_H, D_H),
]
WOFF = {}
_off = 0
for _n, _k, _nn in WSPEC:
    WOFF[_n] = (_off, _k // P, _nn)
    _off += (_k // P) * _nn
WALL = _off

# accumulator columns: spatial (8) | temporal (2) | energy (4)
SP0, TM0, EN0, NACC = 0, 8, 10, 14

_CACHE = {}
LAST_RESULTS = None


def _mm(nc, ps, terms):
    n = len(terms)
    for i, (lhsT, rhs) in enumerate(terms):
        nc.tensor.matmul(ps, lhsT, rhs, start=(i == 0), stop=(i == n - 1))


def build():
    nc = bacc.Bacc(target_bir_lowering=False, trn_type="TRN2", debug=False)

    def din(name, shape, dt=f32):
        return nc.dram_tensor(name, shape, dt, kind="ExternalInput").ap()

    def dout(name, shape):
        return nc.dram_tensor(name, shape, f32, kind="ExternalOutput").ap()

    d_xI = din("xI", [D_IN, BC], f32r)
    # packed rows: xh 0:256 | xz 256:512 | xez 512:768 | xeh 768:1024
    d_xsm = din("xsm", [4 * D_Z, BC], f32r)
    d_xth = din("xth", [D_T, BC])            # 0.5 * theta.T
    d_wall = din("w_all", [P, WALL], f32r)   # packed lhsT weights
    d_wt2z = din("w_t2z", [D_T, D_Z], f32r)  # 0.02 * relu(W_t2z).T

    d_ihat = dout("o_ihat", [D_IN, BC])
    d_oz = dout("o_z", [D_Z, BC])
    d_osm = dout("o_sm", [2 * D_Z, BC])      # z_hat 0:256 | h 256:512
    d_acc = dout("o_acc", [P, NACC])

    r_xI = d_xI.rearrange("(a p) n -> p a n", p=P)
    r_xsm = d_xsm.rearrange("(a p) n -> p a n", p=P)
    r_oz = d_oz.rearrange("(a p) n -> p a n", p=P)
    r_osm = d_osm.rearrange("(a p) n -> p a n", p=P)

    def hs(h):
        return slice(h * HF, (h + 1) * HF)

    with tile.TileContext(nc) as tc:
        with (
            tc.tile_pool(name="wp", bufs=1) as wp,
            tc.tile_pool(name="iop", bufs=1) as iop,
            tc.tile_pool(name="mid", bufs=1) as mid,
            tc.tile_pool(name="sc", bufs=2) as sc,
            tc.tile_pool(name="ihp", bufs=2) as ihp,
            tc.tile_pool(name="accp", bufs=1) as accp,
            tc.tile_pool(name="ps", bufs=8, space=bass.MemorySpace.PSUM) as psp,
        ):
            # ---- loads ----
            wall = wp.tile([P, WALL], f32r, tag="wall")
            nc.sync.dma_start(wall[:], d_wall[:])
            wt2z = wp.tile([D_T, D_Z], f32r, tag="wt2z")
            nc.sync.dma_start(wt2z[:], d_wt2z[:])

            def w(name):
                off, a, n = WOFF[name]
                return wall[:, off:off + a * n].rearrange("p (a n) -> p a n", n=n)

            xsm = iop.tile([P, 8, BC], f32r, tag="xsm")
            nc.sync.dma_start(xsm[:, 0:4, :], r_xsm[:, 0:4, :])   # xh, xz
            nc.sync.dma_start(xsm[:, 4:8, :], r_xsm[:, 4:8, :])   # xez, xeh
            xth = iop.tile([D_T, BC], f32, tag="xth")
            nc.sync.dma_start(xth[:], d_xth[:])
            xI = iop.tile([P, KI, BC], f32r, tag="xI")
            nc.sync.dma_start(xI[:, 0:4, :], r_xI[:, 0:4, :])
            nc.sync.dma_start(xI[:, 4:8, :], r_xI[:, 4:8, :])

            xh = xsm[:, 0:KZ, :]
            xz = xsm[:, KZ:2 * KZ, :]
            xez = xsm[:, 2 * KZ:3 * KZ, :].bitcast(f32)
            xeh = xsm[:, 3 * KZ:4 * KZ, :].bitcast(f32)

            acc = accp.tile([P, NACC], f32, tag="acc")
            nc.vector.memset(acc[:], 0.0)

            # ---- prior: sigma_p raw softplus (Exp per psum bank, one big Ln) ----
            sp_e = sc.tile([P, KZ, BC], f32, tag="sc8")
            for n in range(KZ):
                for h in range(NH):
                    ps = psp.tile([P, HF], f32, tag="ps")
                    _mm(nc, ps[:], [(w("plv")[:, k, bass.ts(n, P)], xh[:, k, hs(h)])
                                    for k in range(KZ)])
                    nc.scalar.activation(sp_e[:, n, hs(h)], ps[:], Act.Exp, scale=1.2)
            sp = mid.tile([P, KZ, BC], f32r, tag="sp")
            nc.scalar.activation(sp[:], sp_e[:], Act.Ln, bias=1.0)

            mu_p = mid.tile([P, KZ, BC], f32, tag="mu_p")
            for n in range(KZ):
                for h in range(NH):
                    ps = psp.tile([P, HF], f32, tag="ps")
                    _mm(nc, ps[:], [(w("pmu")[:, k, bass.ts(n, P)], xh[:, k, hs(h)])
                                    for k in range(KZ)])
                    nc.scalar.activation(mu_p[:, n, hs(h)], ps[:], Act.Relu)

            # ---- theta ----
            th_s = mid.tile([D_T, BC], f32, tag="th_s")
            for h in range(NH):
                ps_th = psp.tile([D_T, HF], f32, tag="ps")
                _mm(nc, ps_th[:],
                    [(w("i2t")[:, k, :], xI[:, k, hs(h)]) for k in range(KI)]
                    + [(w("vip")[:, k, :], sp[:, k, hs(h)]) for k in range(KZ)])
                nc.vector.tensor_add(th_s[:, hs(h)], ps_th[:], xth[:, hs(h)])
            nc.scalar.activation(th_s[:], th_s[:], Act.Exp, scale=0.5)
            theta = mid.tile([D_T, BC], f32r, tag="theta")
            nc.scalar.activation(theta[:], th_s[:], Act.Ln, bias=1.0)

            # ---- posterior ----
            mu_q = mid.tile([P, KZ, BC], f32, tag="mu_q")
            for n in range(KZ):
                for h in range(NH):
                    ps = psp.tile([P, HF], f32, tag="ps")
                    _mm(nc, ps[:], [(w("qmu")[:, k, bass.ts(n, P)], xI[:, k, hs(h)])
                                    for k in range(KI)])
                    nc.scalar.activation(mu_q[:, n, hs(h)], ps[:], Act.Relu)
            sq = mid.tile([P, KZ, BC], f32, tag="sq")
            for n in range(KZ):
                for h in range(NH):
                    ps = psp.tile([P, HF], f32, tag="ps")
                    _mm(nc, ps[:], [(w("qlv")[:, k, bass.ts(n, P)], xI[:, k, hs(h)])
                                    for k in range(KI)])
                    nc.scalar.activation(sq[:, n, hs(h)], ps[:], Act.Relu)

            # ---- higher state h (PE filler; evicts into osm[2:4]) ----
            osm = mid.tile([P, 2 * KZ, BC], f32, tag="osm")
            for n in range(KZ):
                for h in range(NH):
                    ps = psp.tile([P, HF], f32, tag="ps")
                    _mm(nc, ps[:],
                        [(w("z2h")[:, k, bass.ts(n, P)], xz[:, k, hs(h)])
                         for k in range(KZ)]
                        + [(w("h2h")[:, k, bass.ts(n, P)], xh[:, k, hs(h)])
                           for k in range(KZ)])
                    nc.scalar.activation(osm[:, KZ + n, hs(h)], ps[:], Act.Relu)

            # ---- z = relu(min(mu_q + eps_z*sigma_q, 1) - thr) ----
            t0 = sc.tile([P, KZ, BC], f32, tag="sc8")
            nc.vector.tensor_mul(t0[:], xez, sq[:])
            nc.vector.tensor_add(t0[:], t0[:], mu_q[:])
            z = mid.tile([P, KZ, BC], f32r, tag="z")
            for n in range(KZ):
                for h in range(NH):
                    ps = psp.tile([P, HF], f32, tag="ps")
                    _mm(nc, ps[:], [(wt2z[:, bass.ts(n, P)], theta[:, hs(h)])])
                    nc.vector.scalar_tensor_tensor(
                        t0[:, n, hs(h)], t0[:, n, hs(h)], 1.0, ps[:],
                        op0=Alu.min, op1=Alu.subtract)
                    nc.vector.tensor_scalar(
                        z[:, n, hs(h)], t0[:, n, hs(h)], 0.0, None,
                        op0=Alu.max, op1=Alu.add,
                        accum_out=acc[:, EN0 + n * NH + h: EN0 + n * NH + h + 1])
            nc.sync.dma_start(r_oz[:], z[:].bitcast(f32))

            # ---- z_hat (into osm[0:2]) + temporal partials ----
            m0 = sc.tile([P, KZ, BC], f32, tag="sc8")
            nc.vector.tensor_mul(m0[:], xeh, sp[:].bitcast(f32))
            nc.vector.tensor_add(osm[:, 0:KZ, :], m0[:], mu_p[:])
            nc.vector.tensor_sub(m0[:], z[:].bitcast(f32), osm[:, 0:KZ, :])
            nc.vector.tensor_mul(m0[:], m0[:], m0[:])
            nc.vector.tensor_reduce(acc[:, TM0:TM0 + KZ], m0[:],
                                    axis=mybir.AxisListType.X, op=Alu.add)

            # ---- reconstruction r1 ----
            r1 = mid.tile([P, KZ, BC], f32r, tag="r1")
            for n in range(KZ):
                for h in range(NH):
                    ps = psp.tile([P, HF], f32, tag="ps")
                    _mm(nc, ps[:], [(w("r1")[:, k, bass.ts(n, P)], z[:, k, hs(h)])
                                    for k in range(KZ)])
                    nc.vector.tensor_copy(r1[:, n, hs(h)], ps[:])

            # ---- I_hat = sigmoid(r1 @ W_rec2.T); spatial partials on GpSimd ----
            for n in range(KI):
                ih = ihp.tile([P, BC], f32, tag="ih")
                for h in range(NH):
                    ps = psp.tile([P, HF], f32, tag="ps")
                    _mm(nc, ps[:], [(w("r2")[:, k, bass.ts(n, P)], r1[:, k, hs(h)])
                                    for k in range(KZ)])
                    nc.scalar.activation(ih[:, hs(h)], ps[:], Act.Sigmoid)
                nc.sync.dma_start(d_ihat[n * P:(n + 1) * P, :], ih[:])
                nc.gpsimd.tensor_sub(ih[:], xI[:, n, :].bitcast(f32), ih[:])
                nc.gpsimd.tensor_mul(ih[:], ih[:], ih[:])
                nc.vector.tensor_reduce(acc[:, SP0 + n: SP0 + n + 1], ih[:],
                                        axis=mybir.AxisListType.X, op=Alu.add)

            nc.sync.dma_start(r_osm[:], osm[:])
            nc.sync.dma_start(d_acc[:], acc[:])

    nc.compile()
    return nc


def prep_inputs(inputs):
    """Host-side shard + transpose + scale/packing. Returns in_maps (8 cores)."""
    ac = np.ascontiguousarray

    def g(name):
        return np.asarray(inputs[name], dtype=np.float32)

    I_t, h_m_1, z_m_1 = g("I_t"), g("h_m_1"), g("z_m_1")
    theta_m_1, eps_z, eps_zhat = g("theta_m_1"), g("eps_z"), g("eps_zhat")

    def pack_w(wt):  # wt: [K, N] lhsT -> [128, K//128 * N]
        K, N = wt.shape
        return wt.reshape(K // P, P, N).transpose(1, 0, 2).reshape(P, -1)

    wparts = {
        "pmu": g("W_prior_mu").T,
        "plv": g("W_prior_lv").T,
        "i2t": 0.1 * g("W_i2t").T,
        "vip": -np.maximum(g("W_vip2t"), 0.0).T / np.float32(1.2),
        "qmu": g("W_post_mu").T,
        "qlv": g("W_post_lv").T,
        "r1": g("W_rec1").T,
        "r2": g("W_rec2").T,
        "z2h": g("W_z2h").T,
        "h2h": g("W_h2h").T,
    }
    w_all = np.concatenate(
        [pack_w(ac(wparts[n].astype(np.float32))) for n, _, _ in WSPEC], axis=1)
    w_all = ac(w_all)
    w_t2z = ac(0.02 * np.maximum(g("W_t2z"), 0.0).T)

    in_maps = []
    for i in range(N_CORES):
        r = slice(i * BC, (i + 1) * BC)
        xsm = np.concatenate([
            h_m_1[r].T, z_m_1[r].T, eps_z[r].T,
            eps_zhat[r].T / np.float32(1.2),
        ], axis=0)
        in_maps.append({
            "xI": ac(I_t[r].T),
            "xsm": ac(xsm),
            "xth": ac(0.5 * theta_m_1[r].T),
            "w_all": w_all,
            "w_t2z": w_t2z,
        })
    return in_maps


def gather_outputs(results):
    I_hat = np.empty((B, D_IN), np.float32)
    z = np.empty((B, D_Z), np.float32)
    h = np.empty((B, D_Z), np.float32)
    z_hat = np.empty((B, D_Z), np.float32)
    sp_sum = tm_sum = en_sum = 0.0
    for i, res in enumerate(results):
        r = slice(i * BC, (i + 1) * BC)
        I_hat[r] = res["o_ihat"].T
        z[r] = res["o_z"].T
        osm = res["o_sm"]
        z_hat[r] = osm[:D_Z].T
        h[r] = osm[D_Z:].T
        a = res["o_acc"].astype(np.float64)
        sp_sum += a[:, SP0:TM0].sum()
        tm_sum += a[:, TM0:EN0].sum()
        en_sum += a[:, EN0:NACC].sum()
    spatial = np.float32(sp_sum / (B * D_IN))
    temporal = np.float32(tm_sum / (B * D_Z))
    energy = np.float32(en_sum / (B * D_Z))
    return (I_hat, z, h, z_hat, spatial, temporal, energy)


def kernel(**inputs):
    global LAST_RESULTS
    if "nc" not in _CACHE:
        _CACHE["nc"] = build()
    nc = _CACHE["nc"]
    in_maps = prep_inputs(inputs)
    trace = bool(os.environ.get("KERNEL_TRACE"))
    res = run_bass_kernel_spmd(nc, in_maps, core_ids=list(range(N_CORES)),
                               trace=trace)
    LAST_RESULTS = res
    return gather_outputs(res.results)


# revision 12
# speedup vs baseline: 1.1597x; 1.1597x over previous
"""Trainium2 Bass kernel for the EnergyConstrainedPredictiveCodingModel.

Pure data parallel: batch (8192) sharded across 8 NeuronCores (1024 rows
each); weights replicated. Everything on-device is kept in transposed layout
[feature, batch] so the PE array contracts along the partition dim:

    Y.T[N, B] = W @ X.T  ==  matmul(out, lhsT=W.T[K, N], rhs=X.T[K, B])

Matmuls run in float32r (full-rate fp32 PE path). Reference scale factors
(0.1 i2t, -1/1.2 vip, 0.02 t2z, 0.5 theta, eps_zhat/1.2) are folded into
host-side preprocessing. softplus = Ln(Exp(x)+1) on ACT; the kernel is
structured so ACT needs exactly two table sets (natural_log_exp, sigmoid).
Losses are reduced on-chip into per-partition partials (acc) and finished on
the host. Small weights are packed into one DRAM tensor; xh/xz/xez/xeh are
packed into one; z_hat/h into one output.
"""

import os

import numpy as np

import concourse.bacc as bacc
import concourse.bass as bass
import concourse.tile as tile
from concourse import mybir
from concourse.bass_utils import run_bass_kernel_spmd

f32 = mybir.dt.float32
f32r = mybir.dt.float32r
Alu = mybir.AluOpType
Act = mybir.ActivationFunctionType

N_CORES = 8
B = 8192
BC = B // N_CORES  # 1024 batch rows per core
D_IN, D_Z, D_H, D_T, D_REC = 1024, 256, 256, 16, 256
P = 128
KI = D_IN // P   # 8 k-tiles for D_IN
KZ = D_Z // P    # 2 k-tiles for D_Z / D_H / D_REC
HF = 512         # matmul moving-dim half (fp32 max 512)
NH = BC // HF    # 2 halves

# packed-weight layout: name -> (K, N); stored as [128, K//128 * N] per weight
WSPEC = [
    ("pmu", D_H, D_Z), ("plv", D_H, D_Z), ("i2t", D_IN, D_T),
    ("vip", D_Z, D_T), ("qmu", D_IN, D_Z), ("qlv", D_IN, D_Z),
    ("r1", D_Z, D_REC), ("r2", D_REC, D_IN), ("z2h", D_Z, D_H),
    ("h2h", D_H, D_H),
]
WOFF = {}
_off = 0
for _n, _k, _nn in WSPEC:
    WOFF[_n] = (_off, _k // P, _nn)
    _off += (_k // P) * _nn
WALL = _off

# accumulator columns: spatial (8) | temporal (2) | energy (4)
SP0, TM0, EN0, NACC = 0, 8, 10, 14

_CACHE = {}
LAST_RESULTS = None


def _mm(nc, ps, terms):
    n = len(terms)
    for i, (lhsT, rhs) in enumerate(terms):
        nc.tensor.matmul(ps, lhsT, rhs, start=(i == 0), stop=(i == n - 1))


def build():
    nc = bacc.Bacc(target_bir_lowering=False, trn_type="TRN2", debug=False)

    def din(name, shape, dt=f32):
        return nc.dram_tensor(name, shape, dt, kind="ExternalInput").ap()

    def dout(name, shape):
        return nc.dram_tensor(name, shape, f32, kind="ExternalOutput").ap()

    d_xI = din("xI", [D_IN, BC], f32r)
    # packed rows: xh 0:256 | xz 256:512 | xez 512:768 | xeh 768:1024
    d_xsm = din("xsm", [4 * D_Z, BC], f32r)
    d_xth = din("xth", [D_T, BC])            # 0.5 * theta.T
    d_wall = din("w_all", [P, WALL], f32r)   # packed lhsT weights
    d_wt2z = din("w_t2z", [D_T, D_Z], f32r)  # 0.02 * relu(W_t2z).T

    d_ihat = dout("o_ihat", [D_IN, BC])
    d_oz = dout("o_z", [D_Z, BC])
    d_osm = dout("o_sm", [2 * D_Z, BC])      # z_hat 0:256 | h 256:512
    d_acc = dout("o_acc", [P, NACC])

    r_xI = d_xI.rearrange("(a p) n -> p a n", p=P)
    r_xsm = d_xsm.rearrange("(a p) n -> p a n", p=P)
    r_oz = d_oz.rearrange("(a p) n -> p a n", p=P)
    r_osm = d_osm.rearrange("(a p) n -> p a n", p=P)

    def hs(h):
        return slice(h * HF, (h + 1) * HF)

    with tile.TileContext(nc) as tc:
        with (
            tc.tile_pool(name="wp", bufs=1) as wp,
            tc.tile_pool(name="iop", bufs=1) as iop,
            tc.tile_pool(name="mid", bufs=1) as mid,
            tc.tile_pool(name="sc", bufs=2) as sc,
            tc.tile_pool(name="ihp", bufs=2) as ihp,
            tc.tile_pool(name="accp", bufs=1) as accp,
            tc.tile_pool(name="ps", bufs=8, space=bass.MemorySpace.PSUM) as psp,
        ):
            # ---- loads, issued in need-order so the first matmuls
            # ---- (prior path: w_plv/w_pmu x xh) start within a few us ----
            wall = wp.tile([P, WALL], f32r, tag="wall")
            xsm = iop.tile([P, 8, BC], f32r, tag="xsm")
            xth = iop.tile([D_T, BC], f32, tag="xth")
            xI = iop.tile([P, KI, BC], f32r, tag="xI")
            wt2z = wp.tile([D_T, D_Z], f32r, tag="wt2z")

            def wdma(*names):
                o0, a0, n0 = WOFF[names[0]]
                oL, aL, nL = WOFF[names[-1]]
                lo, hi = o0, oL + aL * nL
                nc.sync.dma_start(wall[:, lo:hi], d_wall[:, lo:hi])

            nc.sync.dma_start(xsm[:, 0:4, :], r_xsm[:, 0:4, :])   # xh, xz
            wdma("pmu", "plv")
            nc.sync.dma_start(xth[:], d_xth[:])
            nc.sync.dma_start(xI[:, 0:4, :], r_xI[:, 0:4, :])
            wdma("i2t", "vip")
            wdma("qmu")
            nc.sync.dma_start(xI[:, 4:8, :], r_xI[:, 4:8, :])
            wdma("qlv")
            nc.sync.dma_start(wt2z[:], d_wt2z[:])
            wdma("r1", "h2h")  # r1, r2, z2h, h2h contiguous in WSPEC
            nc.sync.dma_start(xsm[:, 4:8, :], r_xsm[:, 4:8, :])   # xez, xeh

            def w(name):
                off, a, n = WOFF[name]
                return wall[:, off:off + a * n].rearrange("p (a n) -> p a n", n=n)

            xh = xsm[:, 0:KZ, :]
            xz = xsm[:, KZ:2 * KZ, :]
            xez = xsm[:, 2 * KZ:3 * KZ, :].bitcast(f32)
            xeh = xsm[:, 3 * KZ:4 * KZ, :].bitcast(f32)

            acc = accp.tile([P, NACC], f32, tag="acc")
            nc.vector.memset(acc[:], 0.0)

            # ---- prior: sigma_p raw softplus (Exp per psum bank, one big Ln) ----
            sp_e = sc.tile([P, KZ, BC], f32, tag="sc8")
            for n in range(KZ):
                for h in range(NH):
                    ps = psp.tile([P, HF], f32, tag="ps")
                    _mm(nc, ps[:], [(w("plv")[:, k, bass.ts(n, P)], xh[:, k, hs(h)])
                                    for k in range(KZ)])
                    nc.scalar.activation(sp_e[:, n, hs(h)], ps[:], Act.Exp, scale=1.2)
            sp = mid.tile([P, KZ, BC], f32r, tag="sp")
            nc.scalar.activation(sp[:], sp_e[:], Act.Ln, bias=1.0)

            mu_p = mid.tile([P, KZ, BC], f32, tag="mu_p")
            for n in range(KZ):
                for h in range(NH):
                    ps = psp.tile([P, HF], f32, tag="ps")
                    _mm(nc, ps[:], [(w("pmu")[:, k, bass.ts(n, P)], xh[:, k, hs(h)])
                                    for k in range(KZ)])
                    nc.scalar.activation(mu_p[:, n, hs(h)], ps[:], Act.Relu)

            # ---- theta ----
            th_s = mid.tile([D_T, BC], f32, tag="th_s")
            for h in range(NH):
                ps_th = psp.tile([D_T, HF], f32, tag="ps")
                _mm(nc, ps_th[:],
                    [(w("i2t")[:, k, :], xI[:, k, hs(h)]) for k in range(KI)]
                    + [(w("vip")[:, k, :], sp[:, k, hs(h)]) for k in range(KZ)])
                nc.vector.tensor_add(th_s[:, hs(h)], ps_th[:], xth[:, hs(h)])
            nc.scalar.activation(th_s[:], th_s[:], Act.Exp, scale=0.5)
            theta = mid.tile([D_T, BC], f32r, tag="theta")
            nc.scalar.activation(theta[:], th_s[:], Act.Ln, bias=1.0)

            # ---- posterior ----
            mu_q = mid.tile([P, KZ, BC], f32, tag="mu_q")
            for n in range(KZ):
                for h in range(NH):
                    ps = psp.tile([P, HF], f32, tag="ps")
                    _mm(nc, ps[:], [(w("qmu")[:, k, bass.ts(n, P)], xI[:, k, hs(h)])
                                    for k in range(KI)])
                    nc.scalar.activation(mu_q[:, n, hs(h)], ps[:], Act.Relu)
            sq = mid.tile([P, KZ, BC], f32, tag="sq")
            for n in range(KZ):
                for h in range(NH):
                    ps = psp.tile([P, HF], f32, tag="ps")
                    _mm(nc, ps[:], [(w("qlv")[:, k, bass.ts(n, P)], xI[:, k, hs(h)])
                                    for k in range(KI)])
                    nc.scalar.activation(sq[:, n, hs(h)], ps[:], Act.Relu)

            # ---- higher state h (PE filler; evicts into osm[2:4]) ----
            osm = mid.tile([P, 2 * KZ, BC], f32, tag="osm")
            for n in range(KZ):
                for h in range(NH):
                    ps = psp.tile([P, HF], f32, tag="ps")
                    _mm(nc, ps[:],
                        [(w("z2h")[:, k, bass.ts(n, P)], xz[:, k, hs(h)])
                         for k in range(KZ)]
                        + [(w("h2h")[:, k, bass.ts(n, P)], xh[:, k, hs(h)])
                           for k in range(KZ)])
                    nc.scalar.activation(osm[:, KZ + n, hs(h)], ps[:], Act.Relu)

            # ---- z = relu(min(mu_q + eps_z*sigma_q, 1) - thr) ----
            t0 = sc.tile([P, KZ, BC], f32, tag="sc8")
            nc.vector.tensor_mul(t0[:], xez, sq[:])
            nc.vector.tensor_add(t0[:], t0[:], mu_q[:])
            z = mid.tile([P, KZ, BC], f32r, tag="z")
            for n in range(KZ):
                for h in range(NH):
                    ps = psp.tile([P, HF], f32, tag="ps")
                    _mm(nc, ps[:], [(wt2z[:, bass.ts(n, P)], theta[:, hs(h)])])
                    nc.vector.scalar_tensor_tensor(
                        t0[:, n, hs(h)], t0[:, n, hs(h)], 1.0, ps[:],
                        op0=Alu.min, op1=Alu.subtract)
                    nc.vector.tensor_scalar(
                        z[:, n, hs(h)], t0[:, n, hs(h)], 0.0, None,
                        op0=Alu.max, op1=Alu.add,
                        accum_out=acc[:, EN0 + n * NH + h: EN0 + n * NH + h + 1])
            nc.sync.dma_start(r_oz[:], z[:].bitcast(f32))

            # ---- z_hat (into osm[0:2]) + temporal partials ----
            m0 = sc.tile([P, KZ, BC], f32, tag="sc8")
            nc.vector.tensor_mul(m0[:], xeh, sp[:].bitcast(f32))
            nc.vector.tensor_add(osm[:, 0:KZ, :], m0[:], mu_p[:])
            nc.vector.tensor_sub(m0[:], z[:].bitcast(f32), osm[:, 0:KZ, :])
            nc.vector.tensor_mul(m0[:], m0[:], m0[:])
            nc.vector.tensor_reduce(acc[:, TM0:TM0 + KZ], m0[:],
                                    axis=mybir.AxisListType.X, op=Alu.add)

            # ---- reconstruction r1 ----
            r1 = mid.tile([P, KZ, BC], f32r, tag="r1")
            for n in range(KZ):
                for h in range(NH):
                    ps = psp.tile([P, HF], f32, tag="ps")
                    _mm(nc, ps[:], [(w("r1")[:, k, bass.ts(n, P)], z[:, k, hs(h)])
                                    for k in range(KZ)])
                    nc.vector.tensor_copy(r1[:, n, hs(h)], ps[:])

            # ---- I_hat = sigmoid(r1 @ W_rec2.T); spatial partials on GpSimd ----
            for n in range(KI):
                ih = ihp.tile([P, BC], f32, tag="ih")
                for h in range(NH):
                    ps = psp.tile([P, HF], f32, tag="ps")
                    _mm(nc, ps[:], [(w("r2")[:, k, bass.ts(n, P)], r1[:, k, hs(h)])
                                    for k in range(KZ)])
                    nc.scalar.activation(ih[:, hs(h)], ps[:], Act.Sigmoid)
                nc.sync.dma_start(d_ihat[n * P:(n + 1) * P, :], ih[:])
                nc.vector.tensor_sub(ih[:], xI[:, n, :].bitcast(f32), ih[:])
                nc.scalar.activation(ih[:], ih[:], Act.Square,
                                     accum_out=acc[:, SP0 + n: SP0 + n + 1])

            nc.sync.dma_start(r_osm[:], osm[:])
            nc.sync.dma_start(d_acc[:], acc[:])

    nc.compile()
    return nc


def prep_inputs(inputs):
    """Host-side shard + transpose + scale/packing. Returns in_maps (8 cores)."""
    ac = np.ascontiguousarray

    def g(name):
        return np.asarray(inputs[name], dtype=np.float32)

    I_t, h_m_1, z_m_1 = g("I_t"), g("h_m_1"), g("z_m_1")
    theta_m_1, eps_z, eps_zhat = g("theta_m_1"), g("eps_z"), g("eps_zhat")

    def pack_w(wt):  # wt: [K, N] lhsT -> [128, K//128 * N]
        K, N = wt.shape
        return wt.reshape(K // P, P, N).transpose(1, 0, 2).reshape(P, -1)

    wparts = {
        "pmu": g("W_prior_mu").T,
        "plv": g("W_prior_lv").T,
        "i2t": 0.1 * g("W_i2t").T,
        "vip": -np.maximum(g("W_vip2t"), 0.0).T / np.float32(1.2),
        "qmu": g("W_post_mu").T,
        "qlv": g("W_post_lv").T,
        "r1": g("W_rec1").T,
        "r2": g("W_rec2").T,
        "z2h": g("W_z2h").T,
        "h2h": g("W_h2h").T,
    }
    w_all = np.concatenate(
        [pack_w(ac(wparts[n].astype(np.float32))) for n, _, _ in WSPEC], axis=1)
    w_all = ac(w_all)
    w_t2z = ac(0.02 * np.maximum(g("W_t2z"), 0.0).T)

    in_maps = []
    for i in range(N_CORES):
        r = slice(i * BC, (i + 1) * BC)
        xsm = np.concatenate([
            h_m_1[r].T, z_m_1[r].T, eps_z[r].T,
            eps_zhat[r].T / np.float32(1.2),
        ], axis=0)
        in_maps.append({
            "xI": ac(I_t[r].T),
            "xsm": ac(xsm),
            "xth": ac(0.5 * theta_m_1[r].T),
            "w_all": w_all,
            "w_t2z": w_t2z,
        })
    return in_maps


def gather_outputs(results):
    I_hat = np.empty((B, D_IN), np.float32)
    z = np.empty((B, D_Z), np.float32)
    h = np.empty((B, D_Z), np.float32)
    z_hat = np.empty((B, D_Z), np.float32)
    sp_sum = tm_sum = en_sum = 0.0
    for i, res in enumerate(results):
        r = slice(i * BC, (i + 1) * BC)
        I_hat[r] = res["o_ihat"].T
        z[r] = res["o_z"].T
        osm = res["o_sm"]
        z_hat[r] = osm[:D_Z].T
        h[r] = osm[D_Z:].T
        a = res["o_acc"].astype(np.float64)
        sp_sum += a[:, SP0:TM0].sum()
        tm_sum += a[:, TM0:EN0].sum()
        en_sum += a[:, EN0:NACC].sum()
    spatial = np.float32(sp_sum / (B * D_IN))
    temporal = np.float32(tm_sum / (B * D_Z))
    energy = np.float32(en_sum / (B * D_Z))
    return (I_hat, z, h, z_hat, spatial, temporal, energy)


def kernel(**inputs):
    global LAST_RESULTS
    if "nc" not in _CACHE:
        _CACHE["nc"] = build()
    nc = _CACHE["nc"]
    in_maps = prep_inputs(inputs)
    trace = bool(os.environ.get("KERNEL_TRACE"))
    res = run_bass_kernel_spmd(nc, in_maps, core_ids=list(range(N_CORES)),
                               trace=trace)
    LAST_RESULTS = res
    return gather_outputs(res.results)


# revision 13
# speedup vs baseline: 1.2616x; 1.0879x over previous
"""Trainium2 Bass kernel for the EnergyConstrainedPredictiveCodingModel.

Pure data parallel: batch (8192) sharded across 8 NeuronCores (1024 rows
each); weights replicated. Everything on-device is kept in transposed layout
[feature, batch] so the PE array contracts along the partition dim:

    Y.T[N, B] = W @ X.T  ==  matmul(out, lhsT=W.T[K, N], rhs=X.T[K, B])

Matmuls run in float32r (full-rate fp32 PE path). Reference scale factors
(0.1 i2t, -1/1.2 vip, 0.02 t2z, 0.5 theta, eps_zhat/1.2) are folded into
host-side preprocessing. softplus = Ln(Exp(x)+1) on ACT; the kernel is
structured so ACT needs exactly two table sets (natural_log_exp, sigmoid).
Losses are reduced on-chip into per-partition partials (acc) and finished on
the host. Small weights are packed into one DRAM tensor; xh/xz/xez/xeh are
packed into one; z_hat/h into one output.
"""

import os

import numpy as np

import concourse.bacc as bacc
import concourse.bass as bass
import concourse.tile as tile
from concourse import mybir
from concourse.bass_utils import run_bass_kernel_spmd

f32 = mybir.dt.float32
f32r = mybir.dt.float32r
Alu = mybir.AluOpType
Act = mybir.ActivationFunctionType

N_CORES = 8
B = 8192
BC = B // N_CORES  # 1024 batch rows per core
D_IN, D_Z, D_H, D_T, D_REC = 1024, 256, 256, 16, 256
P = 128
KI = D_IN // P   # 8 k-tiles for D_IN
KZ = D_Z // P    # 2 k-tiles for D_Z / D_H / D_REC
HF = 512         # matmul moving-dim half (fp32 max 512)
NH = BC // HF    # 2 halves

# packed-weight layout: name -> (K, N); stored as [128, K//128 * N] per weight
WSPEC = [
    ("pmu", D_H, D_Z), ("plv", D_H, D_Z), ("i2t", D_IN, D_T),
    ("vip", D_Z, D_T), ("qmu", D_IN, D_Z), ("qlv", D_IN, D_Z),
    ("r1", D_Z, D_REC), ("r2", D_REC, D_IN), ("z2h", D_Z, D_H),
    ("h2h", D_H, D_H),
]
WOFF = {}
_off = 0
for _n, _k, _nn in WSPEC:
    WOFF[_n] = (_off, _k // P, _nn)
    _off += (_k // P) * _nn
WALL = _off

# accumulator columns: spatial (8) | temporal (2) | energy (4)
SP0, TM0, EN0, NACC = 0, 8, 10, 14

_CACHE = {}
LAST_RESULTS = None


def _mm(nc, ps, terms):
    n = len(terms)
    for i, (lhsT, rhs) in enumerate(terms):
        nc.tensor.matmul(ps, lhsT, rhs, start=(i == 0), stop=(i == n - 1))


def build():
    nc = bacc.Bacc(target_bir_lowering=False, trn_type="TRN2", debug=False)

    def din(name, shape, dt=f32):
        return nc.dram_tensor(name, shape, dt, kind="ExternalInput").ap()

    def dout(name, shape):
        return nc.dram_tensor(name, shape, f32, kind="ExternalOutput").ap()

    d_xI = din("xI", [D_IN, BC], f32r)
    # packed rows: xh 0:256 | xz 256:512 | xez 512:768 | xeh 768:1024
    d_xsm = din("xsm", [4 * D_Z, BC], f32r)
    d_xth = din("xth", [D_T, BC])            # 0.5 * theta.T
    d_wall = din("w_all", [P, WALL], f32r)   # packed lhsT weights
    d_wt2z = din("w_t2z", [D_T, D_Z], f32r)  # 0.02 * relu(W_t2z).T

    d_ihat = dout("o_ihat", [D_IN, BC])
    d_oz = dout("o_z", [D_Z, BC])
    d_osm = dout("o_sm", [2 * D_Z, BC])      # z_hat 0:256 | h 256:512
    d_acc = dout("o_acc", [P, NACC])

    r_xI = d_xI.rearrange("(a p) n -> p a n", p=P)
    r_xsm = d_xsm.rearrange("(a p) n -> p a n", p=P)
    r_oz = d_oz.rearrange("(a p) n -> p a n", p=P)
    r_osm = d_osm.rearrange("(a p) n -> p a n", p=P)

    def hs(h):
        return slice(h * HF, (h + 1) * HF)

    with tile.TileContext(nc) as tc:
        with (
            tc.tile_pool(name="wp", bufs=1) as wp,
            tc.tile_pool(name="iop", bufs=1) as iop,
            tc.tile_pool(name="mid", bufs=1) as mid,
            tc.tile_pool(name="sc", bufs=2) as sc,
            tc.tile_pool(name="ihp", bufs=2) as ihp,
            tc.tile_pool(name="accp", bufs=1) as accp,
            tc.tile_pool(name="ps", bufs=8, space=bass.MemorySpace.PSUM) as psp,
        ):
            # ---- loads, issued in need-order so the first matmuls
            # ---- (prior path: w_plv/w_pmu x xh) start within a few us ----
            wall = wp.tile([P, WALL], f32r, tag="wall")
            xsm = iop.tile([P, 8, BC], f32r, tag="xsm")
            xth = iop.tile([D_T, BC], f32, tag="xth")
            xI = iop.tile([P, KI, BC], f32r, tag="xI")
            wt2z = wp.tile([D_T, D_Z], f32r, tag="wt2z")

            def wdma(*names):
                o0, a0, n0 = WOFF[names[0]]
                oL, aL, nL = WOFF[names[-1]]
                lo, hi = o0, oL + aL * nL
                nc.sync.dma_start(wall[:, lo:hi], d_wall[:, lo:hi])

            wdma("plv")
            nc.sync.dma_start(xsm[:, 0:2, :], r_xsm[:, 0:2, :])   # xh
            wdma("pmu")
            nc.sync.dma_start(xsm[:, 2:4, :], r_xsm[:, 2:4, :])   # xz
            wdma("z2h", "h2h")
            nc.sync.dma_start(xI[:, 0:2, :], r_xI[:, 0:2, :])
            wdma("qmu")
            nc.sync.dma_start(xI[:, 2:4, :], r_xI[:, 2:4, :])
            wdma("qlv")
            nc.sync.dma_start(xI[:, 4:6, :], r_xI[:, 4:6, :])
            nc.sync.dma_start(xI[:, 6:8, :], r_xI[:, 6:8, :])
            wdma("i2t", "vip")
            nc.sync.dma_start(xth[:], d_xth[:])
            nc.sync.dma_start(wt2z[:], d_wt2z[:])
            wdma("r1")
            wdma("r2")
            nc.sync.dma_start(xsm[:, 4:8, :], r_xsm[:, 4:8, :])   # xez, xeh

            def w(name):
                off, a, n = WOFF[name]
                return wall[:, off:off + a * n].rearrange("p (a n) -> p a n", n=n)

            xh = xsm[:, 0:KZ, :]
            xz = xsm[:, KZ:2 * KZ, :]
            xez = xsm[:, 2 * KZ:3 * KZ, :].bitcast(f32)
            xeh = xsm[:, 3 * KZ:4 * KZ, :].bitcast(f32)

            acc = accp.tile([P, NACC], f32, tag="acc")
            nc.vector.memset(acc[:], 0.0)

            # ---- prior: sigma_p raw softplus (Exp per psum bank, one big Ln) ----
            sp_e = sc.tile([P, KZ, BC], f32, tag="sc8")
            for n in range(KZ):
                for h in range(NH):
                    ps = psp.tile([P, HF], f32, tag="ps")
                    _mm(nc, ps[:], [(w("plv")[:, k, bass.ts(n, P)], xh[:, k, hs(h)])
                                    for k in range(KZ)])
                    nc.scalar.activation(sp_e[:, n, hs(h)], ps[:], Act.Exp, scale=1.2)
            sp = mid.tile([P, KZ, BC], f32r, tag="sp")
            nc.scalar.activation(sp[:], sp_e[:], Act.Ln, bias=1.0)

            mu_p = mid.tile([P, KZ, BC], f32, tag="mu_p")
            for n in range(KZ):
                for h in range(NH):
                    ps = psp.tile([P, HF], f32, tag="ps")
                    _mm(nc, ps[:], [(w("pmu")[:, k, bass.ts(n, P)], xh[:, k, hs(h)])
                                    for k in range(KZ)])
                    nc.scalar.activation(mu_p[:, n, hs(h)], ps[:], Act.Relu)

            # ---- higher state h (PE filler; evicts into osm[2:4]) ----
            osm = mid.tile([P, 2 * KZ, BC], f32, tag="osm")
            for n in range(KZ):
                for h in range(NH):
                    ps = psp.tile([P, HF], f32, tag="ps")
                    _mm(nc, ps[:],
                        [(w("z2h")[:, k, bass.ts(n, P)], xz[:, k, hs(h)])
                         for k in range(KZ)]
                        + [(w("h2h")[:, k, bass.ts(n, P)], xh[:, k, hs(h)])
                           for k in range(KZ)])
                    nc.scalar.activation(osm[:, KZ + n, hs(h)], ps[:], Act.Relu)

            # ---- theta ----
            th_s = mid.tile([D_T, BC], f32, tag="th_s")
            for h in range(NH):
                ps_th = psp.tile([D_T, HF], f32, tag="ps")
                _mm(nc, ps_th[:],
                    [(w("i2t")[:, k, :], xI[:, k, hs(h)]) for k in range(KI)]
                    + [(w("vip")[:, k, :], sp[:, k, hs(h)]) for k in range(KZ)])
                nc.vector.tensor_add(th_s[:, hs(h)], ps_th[:], xth[:, hs(h)])
            nc.scalar.activation(th_s[:], th_s[:], Act.Exp, scale=0.5)
            theta = mid.tile([D_T, BC], f32r, tag="theta")
            nc.scalar.activation(theta[:], th_s[:], Act.Ln, bias=1.0)

            # ---- posterior ----
            mu_q = mid.tile([P, KZ, BC], f32, tag="mu_q")
            for n in range(KZ):
                for h in range(NH):
                    ps = psp.tile([P, HF], f32, tag="ps")
                    _mm(nc, ps[:], [(w("qmu")[:, k, bass.ts(n, P)], xI[:, k, hs(h)])
                                    for k in range(KI)])
                    nc.scalar.activation(mu_q[:, n, hs(h)], ps[:], Act.Relu)
            sq = mid.tile([P, KZ, BC], f32, tag="sq")
            for n in range(KZ):
                for h in range(NH):
                    ps = psp.tile([P, HF], f32, tag="ps")
                    _mm(nc, ps[:], [(w("qlv")[:, k, bass.ts(n, P)], xI[:, k, hs(h)])
                                    for k in range(KI)])
                    nc.scalar.activation(sq[:, n, hs(h)], ps[:], Act.Relu)

            # ---- z = relu(min(mu_q + eps_z*sigma_q, 1) - thr) ----
            t0 = sc.tile([P, KZ, BC], f32, tag="sc8")
            nc.vector.tensor_mul(t0[:], xez, sq[:])
            nc.vector.tensor_add(t0[:], t0[:], mu_q[:])
            z = mid.tile([P, KZ, BC], f32r, tag="z")
            for n in range(KZ):
                for h in range(NH):
                    ps = psp.tile([P, HF], f32, tag="ps")
                    _mm(nc, ps[:], [(wt2z[:, bass.ts(n, P)], theta[:, hs(h)])])
                    nc.vector.scalar_tensor_tensor(
                        t0[:, n, hs(h)], t0[:, n, hs(h)], 1.0, ps[:],
                        op0=Alu.min, op1=Alu.subtract)
                    nc.vector.tensor_scalar(
                        z[:, n, hs(h)], t0[:, n, hs(h)], 0.0, None,
                        op0=Alu.max, op1=Alu.add,
                        accum_out=acc[:, EN0 + n * NH + h: EN0 + n * NH + h + 1])
            nc.gpsimd.dma_start(r_oz[:], z[:].bitcast(f32))

            # ---- z_hat (into osm[0:2]) + temporal partials ----
            m0 = sc.tile([P, KZ, BC], f32, tag="sc8")
            nc.vector.tensor_mul(m0[:], xeh, sp[:].bitcast(f32))
            nc.vector.tensor_add(osm[:, 0:KZ, :], m0[:], mu_p[:])
            nc.vector.tensor_sub(m0[:], z[:].bitcast(f32), osm[:, 0:KZ, :])
            nc.vector.tensor_mul(m0[:], m0[:], m0[:])
            nc.vector.tensor_reduce(acc[:, TM0:TM0 + KZ], m0[:],
                                    axis=mybir.AxisListType.X, op=Alu.add)

            # ---- reconstruction r1 ----
            r1 = mid.tile([P, KZ, BC], f32r, tag="r1")
            for n in range(KZ):
                for h in range(NH):
                    ps = psp.tile([P, HF], f32, tag="ps")
                    _mm(nc, ps[:], [(w("r1")[:, k, bass.ts(n, P)], z[:, k, hs(h)])
                                    for k in range(KZ)])
                    nc.vector.tensor_copy(r1[:, n, hs(h)], ps[:])

            # ---- I_hat = sigmoid(r1 @ W_rec2.T); spatial partials on GpSimd ----
            for n in range(KI):
                ih = ihp.tile([P, BC], f32, tag="ih")
                for h in range(NH):
                    ps = psp.tile([P, HF], f32, tag="ps")
                    _mm(nc, ps[:], [(w("r2")[:, k, bass.ts(n, P)], r1[:, k, hs(h)])
                                    for k in range(KZ)])
                    nc.scalar.activation(ih[:, hs(h)], ps[:], Act.Sigmoid)
                nc.gpsimd.dma_start(d_ihat[n * P:(n + 1) * P, :], ih[:])
                dsp = ihp.tile([P, BC], f32, tag="dsp")
                nc.vector.tensor_sub(dsp[:], xI[:, n, :].bitcast(f32), ih[:])
                nc.scalar.activation(dsp[:], dsp[:], Act.Square,
                                     accum_out=acc[:, SP0 + n: SP0 + n + 1])

            nc.gpsimd.dma_start(r_osm[:], osm[:])
            nc.gpsimd.dma_start(d_acc[:], acc[:])

    nc.compile()
    return nc


def prep_inputs(inputs):
    """Host-side shard + transpose + scale/packing. Returns in_maps (8 cores)."""
    ac = np.ascontiguousarray

    def g(name):
        return np.asarray(inputs[name], dtype=np.float32)

    I_t, h_m_1, z_m_1 = g("I_t"), g("h_m_1"), g("z_m_1")
    theta_m_1, eps_z, eps_zhat = g("theta_m_1"), g("eps_z"), g("eps_zhat")

    def pack_w(wt):  # wt: [K, N] lhsT -> [128, K//128 * N]
        K, N = wt.shape
        return wt.reshape(K // P, P, N).transpose(1, 0, 2).reshape(P, -1)

    wparts = {
        "pmu": g("W_prior_mu").T,
        "plv": g("W_prior_lv").T,
        "i2t": 0.1 * g("W_i2t").T,
        "vip": -np.maximum(g("W_vip2t"), 0.0).T / np.float32(1.2),
        "qmu": g("W_post_mu").T,
        "qlv": g("W_post_lv").T,
        "r1": g("W_rec1").T,
        "r2": g("W_rec2").T,
        "z2h": g("W_z2h").T,
        "h2h": g("W_h2h").T,
    }
    w_all = np.concatenate(
        [pack_w(ac(wparts[n].astype(np.float32))) for n, _, _ in WSPEC], axis=1)
    w_all = ac(w_all)
    w_t2z = ac(0.02 * np.maximum(g("W_t2z"), 0.0).T)

    in_maps = []
    for i in range(N_CORES):
        r = slice(i * BC, (i + 1) * BC)
        xsm = np.concatenate([
            h_m_1[r].T, z_m_1[r].T, eps_z[r].T,
            eps_zhat[r].T / np.float32(1.2),
        ], axis=0)
        in_maps.append({
            "xI": ac(I_t[r].T),
            "xsm": ac(xsm),
            "xth": ac(0.5 * theta_m_1[r].T),
            "w_all": w_all,
            "w_t2z": w_t2z,
        })
    return in_maps


def gather_outputs(results):
    I_hat = np.empty((B, D_IN), np.float32)
    z = np.empty((B, D_Z), np.float32)
    h = np.empty((B, D_Z), np.float32)
    z_hat = np.empty((B, D_Z), np.float32)
    sp_sum = tm_sum = en_sum = 0.0
    for i, res in enumerate(results):
        r = slice(i * BC, (i + 1) * BC)
        I_hat[r] = res["o_ihat"].T
        z[r] = res["o_z"].T
        osm = res["o_sm"]
        z_hat[r] = osm[:D_Z].T
        h[r] = osm[D_Z:].T
        a = res["o_acc"].astype(np.float64)
        sp_sum += a[:, SP0:TM0].sum()
        tm_sum += a[:, TM0:EN0].sum()
        en_sum += a[:, EN0:NACC].sum()
    spatial = np.float32(sp_sum / (B * D_IN))
    temporal = np.float32(tm_sum / (B * D_Z))
    energy = np.float32(en_sum / (B * D_Z))
    return (I_hat, z, h, z_hat, spatial, temporal, energy)


def kernel(**inputs):
    global LAST_RESULTS
    if "nc" not in _CACHE:
        _CACHE["nc"] = build()
    nc = _CACHE["nc"]
    in_maps = prep_inputs(inputs)
    trace = bool(os.environ.get("KERNEL_TRACE"))
    res = run_bass_kernel_spmd(nc, in_maps, core_ids=list(range(N_CORES)),
                               trace=trace)
    LAST_RESULTS = res
    return gather_outputs(res.results)


# revision 14
# speedup vs baseline: 1.2863x; 1.0196x over previous
"""Trainium2 Bass kernel for the EnergyConstrainedPredictiveCodingModel.

Pure data parallel: batch (8192) sharded across 8 NeuronCores (1024 rows
each); weights replicated. Everything on-device is kept in transposed layout
[feature, batch] so the PE array contracts along the partition dim:

    Y.T[N, B] = W @ X.T  ==  matmul(out, lhsT=W.T[K, N], rhs=X.T[K, B])

Matmuls run in float32r (full-rate fp32 PE path). Reference scale factors
(0.1 i2t, -1/1.2 vip, 0.02 t2z, 0.5 theta, eps_zhat/1.2) are folded into
host-side preprocessing. softplus = Ln(Exp(x)+1) on ACT; the kernel is
structured so ACT needs exactly two table sets (natural_log_exp, sigmoid).
Losses are reduced on-chip into per-partition partials (acc) and finished on
the host. Small weights are packed into one DRAM tensor; xh/xz/xez/xeh are
packed into one; z_hat/h into one output.
"""

import os

import numpy as np

import concourse.bacc as bacc
import concourse.bass as bass
import concourse.tile as tile
from concourse import mybir
from concourse.bass_utils import run_bass_kernel_spmd

f32 = mybir.dt.float32
f32r = mybir.dt.float32r
Alu = mybir.AluOpType
Act = mybir.ActivationFunctionType

N_CORES = 8
B = 8192
BC = B // N_CORES  # 1024 batch rows per core
D_IN, D_Z, D_H, D_T, D_REC = 1024, 256, 256, 16, 256
P = 128
KI = D_IN // P   # 8 k-tiles for D_IN
KZ = D_Z // P    # 2 k-tiles for D_Z / D_H / D_REC
HF = 512         # matmul moving-dim half (fp32 max 512)
NH = BC // HF    # 2 halves

# packed-weight layout: name -> (K, N); stored as [128, K//128 * N] per weight
WSPEC = [
    ("pmu", D_H, D_Z), ("plv", D_H, D_Z), ("i2t", D_IN, D_T),
    ("vip", D_Z, D_T), ("qmu", D_IN, D_Z), ("qlv", D_IN, D_Z),
    ("r1", D_Z, D_REC), ("r2", D_REC, D_IN), ("z2h", D_Z, D_H),
    ("h2h", D_H, D_H),
]
WOFF = {}
_off = 0
for _n, _k, _nn in WSPEC:
    WOFF[_n] = (_off, _k // P, _nn)
    _off += (_k // P) * _nn
WALL = _off

# accumulator columns: spatial (8) | temporal (2) | energy (4)
SP0, TM0, EN0, NACC = 0, 8, 10, 14

_CACHE = {}
LAST_RESULTS = None


def _mm(nc, ps, terms):
    n = len(terms)
    for i, (lhsT, rhs) in enumerate(terms):
        nc.tensor.matmul(ps, lhsT, rhs, start=(i == 0), stop=(i == n - 1))


def build():
    nc = bacc.Bacc(target_bir_lowering=False, trn_type="TRN2", debug=False)

    def din(name, shape, dt=f32):
        return nc.dram_tensor(name, shape, dt, kind="ExternalInput").ap()

    def dout(name, shape):
        return nc.dram_tensor(name, shape, f32, kind="ExternalOutput").ap()

    d_xI = din("xI", [D_IN, BC], f32r)
    # packed rows: xh 0:256 | xz 256:512 | xez 512:768 | xeh 768:1024
    d_xsm = din("xsm", [4 * D_Z, BC], f32r)
    d_xth = din("xth", [D_T, BC])            # 0.5 * theta.T
    d_wall = din("w_all", [P, WALL], f32r)   # packed lhsT weights
    d_wt2z = din("w_t2z", [D_T, D_Z], f32r)  # 0.02 * relu(W_t2z).T

    d_ihat = dout("o_ihat", [D_IN, BC])
    d_oz = dout("o_z", [D_Z, BC])
    d_osm = dout("o_sm", [2 * D_Z, BC])      # z_hat 0:256 | h 256:512
    d_acc = dout("o_acc", [P, NACC])

    r_xI = d_xI.rearrange("(a p) n -> p a n", p=P)
    r_xsm = d_xsm.rearrange("(a p) n -> p a n", p=P)
    r_oz = d_oz.rearrange("(a p) n -> p a n", p=P)
    r_osm = d_osm.rearrange("(a p) n -> p a n", p=P)

    def hs(h):
        return slice(h * HF, (h + 1) * HF)

    with tile.TileContext(nc) as tc:
        with (
            tc.tile_pool(name="wp", bufs=1) as wp,
            tc.tile_pool(name="iop", bufs=1) as iop,
            tc.tile_pool(name="mid", bufs=1) as mid,
            tc.tile_pool(name="sc", bufs=2) as sc,
            tc.tile_pool(name="ihp", bufs=2) as ihp,
            tc.tile_pool(name="accp", bufs=1) as accp,
            tc.tile_pool(name="ps", bufs=4, space=bass.MemorySpace.PSUM) as psp,
            tc.tile_pool(name="ps2", bufs=2, space=bass.MemorySpace.PSUM) as ps2p,
        ):
            # ---- loads, issued in need-order so the first matmuls
            # ---- (prior path: w_plv/w_pmu x xh) start within a few us ----
            wall = wp.tile([P, WALL], f32r, tag="wall")
            xsm = iop.tile([P, 8, BC], f32r, tag="xsm")
            xth = iop.tile([D_T, BC], f32, tag="xth")
            xI = iop.tile([P, KI, BC], f32r, tag="xI")
            wt2z = wp.tile([D_T, D_Z], f32r, tag="wt2z")

            def wdma(*names):
                o0, a0, n0 = WOFF[names[0]]
                oL, aL, nL = WOFF[names[-1]]
                lo, hi = o0, oL + aL * nL
                nc.sync.dma_start(wall[:, lo:hi], d_wall[:, lo:hi])

            wdma("plv")
            nc.sync.dma_start(xsm[:, 0:2, :], r_xsm[:, 0:2, :])   # xh
            wdma("pmu")
            nc.sync.dma_start(xsm[:, 2:4, :], r_xsm[:, 2:4, :])   # xz
            wdma("z2h", "h2h")
            nc.sync.dma_start(xI[:, 0:2, :], r_xI[:, 0:2, :])
            wdma("qmu")
            nc.sync.dma_start(xI[:, 2:4, :], r_xI[:, 2:4, :])
            wdma("qlv")
            nc.sync.dma_start(xI[:, 4:6, :], r_xI[:, 4:6, :])
            nc.sync.dma_start(xI[:, 6:8, :], r_xI[:, 6:8, :])
            wdma("i2t", "vip")
            nc.sync.dma_start(xth[:], d_xth[:])
            nc.sync.dma_start(wt2z[:], d_wt2z[:])
            wdma("r1")
            wdma("r2")
            nc.sync.dma_start(xsm[:, 4:8, :], r_xsm[:, 4:8, :])   # xez, xeh

            def w(name):
                off, a, n = WOFF[name]
                return wall[:, off:off + a * n].rearrange("p (a n) -> p a n", n=n)

            xh = xsm[:, 0:KZ, :]
            xz = xsm[:, KZ:2 * KZ, :]
            xez = xsm[:, 2 * KZ:3 * KZ, :].bitcast(f32)
            xeh = xsm[:, 3 * KZ:4 * KZ, :].bitcast(f32)

            acc = accp.tile([P, NACC], f32, tag="acc")
            nc.vector.memset(acc[:], 0.0)

            # ---- prior: sigma_p raw softplus (Exp per psum bank, one big Ln) ----
            sp_e = sc.tile([P, KZ, BC], f32, tag="sc8")
            for n in range(KZ):
                for h in range(NH):
                    ps = psp.tile([P, HF], f32, tag="ps")
                    _mm(nc, ps[:], [(w("plv")[:, k, bass.ts(n, P)], xh[:, k, hs(h)])
                                    for k in range(KZ)])
                    nc.scalar.activation(sp_e[:, n, hs(h)], ps[:], Act.Exp, scale=1.2)
            sp = mid.tile([P, KZ, BC], f32r, tag="sp")
            nc.scalar.activation(sp[:], sp_e[:], Act.Ln, bias=1.0)

            mu_p = mid.tile([P, KZ, BC], f32, tag="mu_p")
            for n in range(KZ):
                for h in range(NH):
                    ps = psp.tile([P, HF], f32, tag="ps")
                    _mm(nc, ps[:], [(w("pmu")[:, k, bass.ts(n, P)], xh[:, k, hs(h)])
                                    for k in range(KZ)])
                    nc.scalar.activation(mu_p[:, n, hs(h)], ps[:], Act.Relu)

            # ---- higher state h (PE filler; evicts into osm[2:4]) ----
            osm = mid.tile([P, 2 * KZ, BC], f32, tag="osm")
            for n in range(KZ):
                for h in range(NH):
                    ps = psp.tile([P, HF], f32, tag="ps")
                    _mm(nc, ps[:],
                        [(w("z2h")[:, k, bass.ts(n, P)], xz[:, k, hs(h)])
                         for k in range(KZ)]
                        + [(w("h2h")[:, k, bass.ts(n, P)], xh[:, k, hs(h)])
                           for k in range(KZ)])
                    nc.scalar.activation(osm[:, KZ + n, hs(h)], ps[:], Act.Relu)

            # ---- theta / posterior / z / r1, pipelined per batch-half so the
            # ---- z chain (DVE+ACT) overlaps the other half's matmuls ----
            th_s = mid.tile([D_T, BC], f32, tag="th_s")
            theta = mid.tile([D_T, BC], f32r, tag="theta")
            mu_q = mid.tile([P, KZ, BC], f32, tag="mu_q")
            sq = mid.tile([P, KZ, BC], f32, tag="sq")
            t0 = sc.tile([P, KZ, BC], f32, tag="sc8")
            z = mid.tile([P, KZ, BC], f32r, tag="z")
            r1 = mid.tile([P, KZ, BC], f32r, tag="r1")
            for h in range(NH):
                # posterior half
                for n in range(KZ):
                    ps = psp.tile([P, HF], f32, tag="ps")
                    _mm(nc, ps[:], [(w("qmu")[:, k, bass.ts(n, P)], xI[:, k, hs(h)])
                                    for k in range(KI)])
                    nc.scalar.activation(mu_q[:, n, hs(h)], ps[:], Act.Relu)
                for n in range(KZ):
                    ps = psp.tile([P, HF], f32, tag="ps")
                    _mm(nc, ps[:], [(w("qlv")[:, k, bass.ts(n, P)], xI[:, k, hs(h)])
                                    for k in range(KI)])
                    nc.scalar.activation(sq[:, n, hs(h)], ps[:], Act.Relu)
                # theta half
                ps_th = psp.tile([D_T, HF], f32, tag="ps")
                _mm(nc, ps_th[:],
                    [(w("i2t")[:, k, :], xI[:, k, hs(h)]) for k in range(KI)]
                    + [(w("vip")[:, k, :], sp[:, k, hs(h)]) for k in range(KZ)])
                nc.vector.tensor_add(th_s[:, hs(h)], ps_th[:], xth[:, hs(h)])
                nc.scalar.activation(th_s[:, hs(h)], th_s[:, hs(h)], Act.Exp,
                                     scale=0.5)
                nc.scalar.activation(theta[:, hs(h)], th_s[:, hs(h)], Act.Ln,
                                     bias=1.0)
                # z half
                nc.vector.tensor_mul(t0[:, :, hs(h)], xez[:, :, hs(h)],
                                     sq[:, :, hs(h)])
                nc.vector.tensor_add(t0[:, :, hs(h)], t0[:, :, hs(h)],
                                     mu_q[:, :, hs(h)])
                for n in range(KZ):
                    ps = psp.tile([P, HF], f32, tag="ps")
                    _mm(nc, ps[:], [(wt2z[:, bass.ts(n, P)], theta[:, hs(h)])])
                    nc.vector.scalar_tensor_tensor(
                        t0[:, n, hs(h)], t0[:, n, hs(h)], 1.0, ps[:],
                        op0=Alu.min, op1=Alu.subtract)
                    nc.vector.tensor_scalar(
                        z[:, n, hs(h)], t0[:, n, hs(h)], 0.0, None,
                        op0=Alu.max, op1=Alu.add,
                        accum_out=acc[:, EN0 + n * NH + h: EN0 + n * NH + h + 1])
                # r1 half
                for n in range(KZ):
                    ps = psp.tile([P, HF], f32, tag="ps")
                    _mm(nc, ps[:], [(w("r1")[:, k, bass.ts(n, P)], z[:, k, hs(h)])
                                    for k in range(KZ)])
                    nc.vector.tensor_copy(r1[:, n, hs(h)], ps[:])
            nc.gpsimd.dma_start(r_oz[:], z[:].bitcast(f32))

            # ---- z_hat (into osm[0:2]) + temporal partials ----
            m0 = sc.tile([P, KZ, BC], f32, tag="sc8")
            nc.vector.tensor_mul(m0[:], xeh, sp[:].bitcast(f32))
            nc.vector.tensor_add(osm[:, 0:KZ, :], m0[:], mu_p[:])
            nc.vector.tensor_sub(m0[:], z[:].bitcast(f32), osm[:, 0:KZ, :])
            nc.vector.tensor_mul(m0[:], m0[:], m0[:])
            nc.vector.tensor_reduce(acc[:, TM0:TM0 + KZ], m0[:],
                                    axis=mybir.AxisListType.X, op=Alu.add)

            # ---- I_hat = sigmoid(r1 @ W_rec2.T); spatial partials on GpSimd ----
            for n in range(KI):
                ih = ihp.tile([P, BC], f32, tag="ih")
                ps2 = ps2p.tile([P, BC], f32, tag="ps2")
                for h in range(NH):
                    _mm(nc, ps2[:, hs(h)],
                        [(w("r2")[:, k, bass.ts(n, P)], r1[:, k, hs(h)])
                         for k in range(KZ)])
                nc.scalar.activation(ih[:], ps2[:], Act.Sigmoid)
                nc.gpsimd.dma_start(d_ihat[n * P:(n + 1) * P, :], ih[:])
                dsp = ihp.tile([P, BC], f32, tag="dsp")
                nc.vector.tensor_sub(dsp[:], xI[:, n, :].bitcast(f32), ih[:])
                nc.scalar.activation(dsp[:], dsp[:], Act.Square,
                                     accum_out=acc[:, SP0 + n: SP0 + n + 1])

            nc.gpsimd.dma_start(r_osm[:], osm[:])
            nc.gpsimd.dma_start(d_acc[:], acc[:])

    nc.compile()
    return nc


def prep_inputs(inputs):
    """Host-side shard + transpose + scale/packing. Returns in_maps (8 cores)."""
    ac = np.ascontiguousarray

    def g(name):
        return np.asarray(inputs[name], dtype=np.float32)

    I_t, h_m_1, z_m_1 = g("I_t"), g("h_m_1"), g("z_m_1")
    theta_m_1, eps_z, eps_zhat = g("theta_m_1"), g("eps_z"), g("eps_zhat")

    def pack_w(wt):  # wt: [K, N] lhsT -> [128, K//128 * N]
        K, N = wt.shape
        return wt.reshape(K // P, P, N).transpose(1, 0, 2).reshape(P, -1)

    wparts = {
        "pmu": g("W_prior_mu").T,
        "plv": g("W_prior_lv").T,
        "i2t": 0.1 * g("W_i2t").T,
        "vip": -np.maximum(g("W_vip2t"), 0.0).T / np.float32(1.2),
        "qmu": g("W_post_mu").T,
        "qlv": g("W_post_lv").T,
        "r1": g("W_rec1").T,
        "r2": g("W_rec2").T,
        "z2h": g("W_z2h").T,
        "h2h": g("W_h2h").T,
    }
    w_all = np.concatenate(
        [pack_w(ac(wparts[n].astype(np.float32))) for n, _, _ in WSPEC], axis=1)
    w_all = ac(w_all)
    w_t2z = ac(0.02 * np.maximum(g("W_t2z"), 0.0).T)

    in_maps = []
    for i in range(N_CORES):
        r = slice(i * BC, (i + 1) * BC)
        xsm = np.concatenate([
            h_m_1[r].T, z_m_1[r].T, eps_z[r].T,
            eps_zhat[r].T / np.float32(1.2),
        ], axis=0)
        in_maps.append({
            "xI": ac(I_t[r].T),
            "xsm": ac(xsm),
            "xth": ac(0.5 * theta_m_1[r].T),
            "w_all": w_all,
            "w_t2z": w_t2z,
        })
    return in_maps


def gather_outputs(results):
    I_hat = np.empty((B, D_IN), np.float32)
    z = np.empty((B, D_Z), np.float32)
    h = np.empty((B, D_Z), np.float32)
    z_hat = np.empty((B, D_Z), np.float32)
    sp_sum = tm_sum = en_sum = 0.0
    for i, res in enumerate(results):
        r = slice(i * BC, (i + 1) * BC)
        I_hat[r] = res["o_ihat"].T
        z[r] = res["o_z"].T
        osm = res["o_sm"]
        z_hat[r] = osm[:D_Z].T
        h[r] = osm[D_Z:].T
        a = res["o_acc"].astype(np.float64)
        sp_sum += a[:, SP0:TM0].sum()
        tm_sum += a[:, TM0:EN0].sum()
        en_sum += a[:, EN0:NACC].sum()
    spatial = np.float32(sp_sum / (B * D_IN))
    temporal = np.float32(tm_sum / (B * D_Z))
    energy = np.float32(en_sum / (B * D_Z))
    return (I_hat, z, h, z_hat, spatial, temporal, energy)


def kernel(**inputs):
    global LAST_RESULTS
    if "nc" not in _CACHE:
        _CACHE["nc"] = build()
    nc = _CACHE["nc"]
    in_maps = prep_inputs(inputs)
    trace = bool(os.environ.get("KERNEL_TRACE"))
    res = run_bass_kernel_spmd(nc, in_maps, core_ids=list(range(N_CORES)),
                               trace=trace)
    LAST_RESULTS = res
    return gather_outputs(res.results)
